# revision 16
# baseline (speedup 1.0000x reference)
"""Causal single-head attention (B=4, T=2048, D=1024) on 8 NeuronCores.

Sharding: 2 cores per batch element. Within a batch, core r (r in {0,1})
handles the strided query rows q_glob = 2*j + r (j = 0..1023). The strided
split makes the causal block structure identical on every core (SPMD-safe)
and balances causal work exactly.

Per-core device program (fp32 PSUM accumulation throughout):
  1. K^T = Wk x^T   fp8e4m3 + DoubleRow (256-deep virtual array, 2x rate);
     Wk is pre-scaled x32 into fp8's normal range
  2. V   = x Wv^T   bf16 (fp8 V quantization passes straight to the output:
     measured 3.6e-2 rel vs 1.2e-2 with QK-only fp8)
  3. Q^T = Wq xq^T  fp8e4m3 + DoubleRow, Wq pre-scaled x32
  4. attention per (query block, 128-key tile), S^T formulation: stationary
     K-tile x moving Q gives S^T[k, q] in PSUM (no separate transpose);
     K^T and Q^T are kept in SBUF as fp8e4m3 so the S^T matmuls also run
     DoubleRow at 2x rate; additive causal mask on the two diagonal tiles;
     exp on ACT (scale 1/(32*1024) absorbs the two x32 weight scales)
     writes E bf16 to SBUF; E is then itself the stationary operand of
     out2 += E^T V, and the softmax denominator accumulates via an extra
     1-column matmul of E against a ones vector (numerator and denominator
     both sum bf16 E, consistently). Final per-row divide split across ACT
     (activation Copy with per-partition scale) and DVE.

DMA discipline: every DMACopy costs ~630ns of globally-serialized HWDGE
occupancy regardless of size, so inputs stream in a handful of large
multi-dim descriptors (2 for each weight, 4 chunks for each x transpose)
ordered so the K-projection's first accumulation groups unblock first.

Host side transposes/casts inputs (x twice: fp8 for QK, bf16 for V) and
de-interleaves outputs.
"""
import orjson
import numpy as np
import ml_dtypes

import concourse.bass as bass
import concourse.mybir as mybir
import concourse.tile as tile
from concourse import bass_utils

B, T, D = 4, 2048, 1024
NCORES = 8
P = 128
JQ = T // 2            # local queries per core (1024)
N_QB = JQ // P         # 8 query blocks of 128
N_IT = D // P          # 8 contraction tiles
N_KT = T // P          # 16 key tiles of 128
KC = 512               # key chunk for S matmuls
N_KC = T // KC         # 4
F32 = mybir.dt.float32
BF16 = mybir.dt.bfloat16
SCALE = 1.0 / 32.0     # 1/sqrt(D)
MASK_NEG = -1.0e9

def _split_waits(blocks):
    """The walrus build in this container accepts at most ONE sync-wait per
    instruction; Tile freely emits several. Split extras onto same-engine
    NoOps inserted immediately before the instruction (engine-serial order
    preserves semantics)."""
    n_split = 0
    for blk in blocks:
        insts = blk.get("instructions", [])
        out = []
        for ins in insts:
            si = ins.get("sync_info")
            waits = (si or {}).get("on_wait") or []
            if len(waits) > 1:
                for i, w in enumerate(waits[:-1]):
                    nop = {
                        "engine": ins["engine"],
                        "ins": [],
                        "name": f"{ins['name']}-w{i}",
                        "opcode": "NoOp",
                        "outs": [],
                        "sync_info": {"on_wait": [w], "on_update": []},
                    }
                    if "debug" in ins:
                        nop["debug"] = ins["debug"]
                    out.append(nop)
                    n_split += 1
                si["on_wait"] = [waits[-1]]
            out.append(ins)
        blk["instructions"] = out
        _split_waits(blk.get("blocks", []) or [])
    return n_split


class _Bass(bass.Bass):
    def to_json_bytes(self):
        d = orjson.loads(super().to_json_bytes())
        for f in d["functions"]:
            _split_waits(f["blocks"])
        return orjson.dumps(d)


def n_kc_of(qb: int) -> int:
    # chunks of 512 keys needed by query block qb (covers q_glob < 256*(qb+1))
    return qb // 2 + 1


# fp8e4m3 + DoubleRow perf mode on the Q/K projections AND the S^T matmuls:
# 2 contraction subtiles per matmul (virtual 256-deep array), 2x PE rate on
# those phases. Wq/Wk are pre-scaled x32 into fp8's normal range; the 32*32
# factor cancels inside the softmax scale. V stays bf16 end-to-end: fp8 V
# errors pass straight to the output (measured 3.6e-2 rel), QK-only is ~1e-2.
FP8_PROJ = True
F8 = mybir.dt.float8e4
W_SCALE = 32.0


def build_nc(fp8_proj: bool = FP8_PROJ) -> bass.Bass:
    nc = _Bass("TRN2", debug=False, num_devices=NCORES)

    QK_DT = F8 if fp8_proj else BF16
    KSTEP = 2 if fp8_proj else 1
    PMODE = mybir.MatmulPerfMode.DoubleRow if fp8_proj else None
    scale_eff = SCALE / (W_SCALE * W_SCALE) if fp8_proj else SCALE

    # each core computes K/V for all T keys from the full x^T
    xhT = nc.dram_tensor("xhT", [D, T], QK_DT, kind="ExternalInput")
    xqT = nc.dram_tensor("xqT", [D, JQ], QK_DT, kind="ExternalInput")
    wqT = nc.dram_tensor("wqT", [D, D], QK_DT, kind="ExternalInput")
    wkT = nc.dram_tensor("wkT", [D, D], QK_DT, kind="ExternalInput")
    wvT = nc.dram_tensor("wvT", [D, D], BF16, kind="ExternalInput")
    if fp8_proj:
        # bf16 copy of x^T for the V projection
        xhTb = nc.dram_tensor("xhTb", [D, T], BF16, kind="ExternalInput")
    maskadd = nc.dram_tensor("maskadd", [2, P, P], F32, kind="ExternalInput")
    # bf16 output (host casts back to f32): halves the store traffic and the
    # output-DMA tail; costs ~0.4% relative quantization, well inside budget
    out = nc.dram_tensor("out", [JQ, D], BF16, kind="ExternalOutput")

    with tile.TileContext(nc) as tc:
        with (
            tc.tile_pool(name="big", bufs=1) as big,
            tc.tile_pool(name="wpool", bufs=1) as wpool,
            tc.tile_pool(name="small", bufs=2) as small,
            tc.tile_pool(name="pwork", bufs=4) as pwork,
            tc.tile_pool(name="mm", bufs=3, space="PSUM") as mm,
            tc.tile_pool(name="denp", bufs=1, space="PSUM") as denp,
            tc.tile_pool(name="o2p", bufs=2, space="PSUM") as o2p,
        ):
            HD = D // 2

            # ---- input DMAs: few large multi-dim descriptors, issue order
            # matches first use: wk cols 0:512, xh chunk 0, wk cols 512:,
            # remaining xh chunks, then V-phase and Q-phase operands.
            wk = wpool.tile([P, N_IT, D], QK_DT, tag="wk")
            wk_r = wkT.rearrange("(it p) o -> p it o", p=P)
            xh = big.tile([P, N_IT, T], QK_DT, tag="xh")
            xh_r = xhT.rearrange("(it p) t -> p it t", p=P)
            # opening pieces split along the contraction (it) axis — full
            # 1KiB DRAM rows per descriptor (column splits would drop the
            # descriptor run below the 512B full-bandwidth threshold) — and
            # interleaved so K-proj matmul #k of the first group unblocks
            # after ~(k+1) pieces; the xh pieces ride the otherwise-idle ACT
            # hwdge queue
            nc.sync.dma_start(wk[:, 0:2, :], wk_r[:, 0:2, :])
            nc.scalar.dma_start(xh[:, 0:4, 0:KC], xh_r[:, 0:4, 0:KC])
            nc.sync.dma_start(wk[:, 2:4, :], wk_r[:, 2:4, :])
            nc.scalar.dma_start(xh[:, 4:N_IT, 0:KC], xh_r[:, 4:N_IT, 0:KC])
            nc.sync.dma_start(wk[:, 4:N_IT, :], wk_r[:, 4:N_IT, :])
            for tc4 in range(1, N_KC):
                nc.sync.dma_start(
                    xh[:, :, tc4 * KC:(tc4 + 1) * KC],
                    xh_r[:, :, tc4 * KC:(tc4 + 1) * KC],
                )

            # ---- resident K^T / V / Q^T for the attention phase ----
            # bf16 masters; fp8 shadows (cast after the source phase) alias
            # the dead xh/xq slots via tag reuse and feed the off-diagonal
            # S^T DoubleRow matmuls. Diagonal tiles (which carry nearly all
            # the attention mass) stay bf16: pure-fp8 S^T measured 1.9e-2
            # rel err, too close to the 2e-2 gate.
            kt_sb = big.tile([P, N_IT, T], BF16, tag="kt")
            v_sb = big.tile([P, N_KT, D], BF16, tag="v")
            qt_sb = big.tile([P, N_IT, JQ], BF16, tag="qt")

            # ---- K^T[o, t] = sum_i WkT[i,o] * xhT[i,t] ----
            # PSUM->SBUF copies alternate ACT/DVE: a DoubleRow accumulation
            # group takes ~430ns of PE but a single copy engine only drains
            # one 512-col tile per ~600ns, which throttled the projections
            for tc4 in range(N_KC):
                for ot in range(N_IT):
                    acc = mm.tile([P, KC], F32, tag="mm512")
                    for it in range(0, N_IT, KSTEP):
                        nc.tensor.matmul(
                            acc[:],
                            wk[:, it:it + KSTEP, ot * P:(ot + 1) * P],
                            xh[:, it:it + KSTEP, tc4 * KC:(tc4 + 1) * KC],
                            start=(it == 0), stop=(it == N_IT - KSTEP),
                            perf_mode=PMODE,
                        )
                    dst = kt_sb[:, ot, tc4 * KC:(tc4 + 1) * KC]
                    if ot % 2 == 0:
                        nc.scalar.activation(
                            dst, acc[:], mybir.ActivationFunctionType.Copy
                        )
                    else:
                        nc.vector.tensor_copy(dst, acc[:])

            if fp8_proj:
                # fp8 shadow of K^T in the (now dead) xh slot; the slot-reuse
                # WAR dependency holds these casts until the last K-proj
                # matmul has read xh. They run on the otherwise-idle GPSIMD
                # engine under the V projection (~70us of slack before the
                # first off-diagonal S^T tile reads them).
                kt8 = big.tile([P, N_IT, T], F8, tag="xh", name="kt8")
                for ot in range(N_IT):
                    nc.gpsimd.tensor_copy(kt8[:, ot, :], kt_sb[:, ot, :])

            # ---- V[t, o] = sum_i xhT[i,t] * WvT[i,o] ----  (bf16)
            wv = wpool.tile([P, N_IT, D], BF16, tag="wv")
            wv_r = wvT.rearrange("(it p) o -> p it o", p=P)
            nc.sync.dma_start(wv[:, :, 0:HD], wv_r[:, :, 0:HD])
            nc.sync.dma_start(wv[:, :, HD:D], wv_r[:, :, HD:D])
            if fp8_proj:
                xhb = big.tile([P, N_IT, T], BF16, tag="xhb")
                xhb_r = xhTb.rearrange("(it p) t -> p it t", p=P)
                for tc4 in range(N_KC):
                    nc.sync.dma_start(
                        xhb[:, :, tc4 * KC:(tc4 + 1) * KC],
                        xhb_r[:, :, tc4 * KC:(tc4 + 1) * KC],
                    )
            else:
                xhb = xh
            xq = big.tile([P, N_IT, JQ], QK_DT, tag="xq")
            nc.sync.dma_start(xq[:], xqT.rearrange("(it p) t -> p it t", p=P))
            for tt in range(N_KT):
                for oc in range(D // KC):
                    acc = mm.tile([P, KC], F32, tag="mm512")
                    for it in range(N_IT):
                        nc.tensor.matmul(
                            acc[:],
                            xhb[:, it, tt * P:(tt + 1) * P],
                            wv[:, it, oc * KC:(oc + 1) * KC],
                            start=(it == 0), stop=(it == N_IT - 1),
                        )
                    nc.vector.tensor_copy(
                        v_sb[:, tt, oc * KC:(oc + 1) * KC], acc[:]
                    )

            # ---- Q^T projection: Q^T[o, j] = sum_i WqT[i,o] * xqT[i,j] ----
            wq = wpool.tile([P, N_IT, D], QK_DT, tag="wq")
            wq_r = wqT.rearrange("(it p) o -> p it o", p=P)
            nc.sync.dma_start(wq[:, :, 0:HD], wq_r[:, :, 0:HD])
            nc.sync.dma_start(wq[:, :, HD:D], wq_r[:, :, HD:D])
            if fp8_proj:
                # fp8 shadow of Q^T (own slot): its DVE casts chase the
                # ACT/DVE projection copies region-by-region, so it is ready
                # when the off-diagonal S^T tiles start. jc-outer so the low
                # query blocks' columns land first (first need is ~qb=1,
                # within ~2us of the attention phase starting).
                qt8 = big.tile([P, N_IT, JQ], F8, tag="qt8")
            for jc in range(JQ // KC):
                for ot in range(N_IT):
                    acc = mm.tile([P, KC], F32, tag="mm512")
                    for it in range(0, N_IT, KSTEP):
                        nc.tensor.matmul(
                            acc[:],
                            wq[:, it:it + KSTEP, ot * P:(ot + 1) * P],
                            xq[:, it:it + KSTEP, jc * KC:(jc + 1) * KC],
                            start=(it == 0), stop=(it == N_IT - KSTEP),
                            perf_mode=PMODE,
                        )
                    dst = qt_sb[:, ot, jc * KC:(jc + 1) * KC]
                    if ot % 2 == 0:
                        nc.scalar.activation(
                            dst, acc[:], mybir.ActivationFunctionType.Copy
                        )
                    else:
                        nc.vector.tensor_copy(dst, acc[:])
                    if fp8_proj:
                        # on GPSIMD: a DVE cast here sits on the PSUM-drain
                        # path and throttles the projection
                        nc.gpsimd.tensor_copy(
                            qt8[:, ot, jc * KC:(jc + 1) * KC], dst
                        )

            # ---- attention (S^T formulation), pipelined over (qb, kt) ----
            # S^T[k, q] per 128-key tile via stationary K-tiles / moving Q
            # (fp8 DoubleRow when enabled); the exp'd tile E then serves
            # directly as the stationary operand of the PV matmuls, so P is
            # never transposed. The softmax denominator accumulates via an
            # extra 1-column matmul against a ones vector on the same
            # stationary.
            masksT = big.tile([P, 2, P], F32, tag="masks")
            # DRAM [2,128,128] -> partition-major per mask
            nc.sync.dma_start(masksT[:], maskadd.rearrange("m p f -> p m f"))
            ones_sb = big.tile([P, 1], BF16, tag="ones")
            nc.any.memset(ones_sb[:], 1.0)

            tiles = [(qb, kt) for qb in range(N_QB) for kt in range(2 * qb + 2)]
            state = {}  # per live qb: out2 + denominator PSUM

            def emit_st_phase(qb, kt):
                if kt == 0:
                    state[qb] = {
                        "out2": o2p.tile([P, D], F32, tag="out2", name="out2"),
                        "den": denp.tile([P, 1], F32, tag="den", name="den"),
                    }
                st = mm.tile([P, P], F32, tag="mm512")
                if fp8_proj and kt < 2 * qb:
                    # off-diagonal: fp8 DoubleRow at 4x the bf16 tile rate
                    for ot in range(0, N_IT, KSTEP):
                        nc.tensor.matmul(
                            st[:],
                            kt8[:, ot:ot + KSTEP, kt * P:(kt + 1) * P],
                            qt8[:, ot:ot + KSTEP, qb * P:(qb + 1) * P],
                            start=(ot == 0), stop=(ot == N_IT - KSTEP),
                            perf_mode=PMODE,
                        )
                else:
                    # diagonal (masked) tiles hold the dominant scores: bf16
                    for ot in range(N_IT):
                        nc.tensor.matmul(
                            st[:],
                            kt_sb[:, ot, kt * P:(kt + 1) * P],
                            qt_sb[:, ot, qb * P:(qb + 1) * P],
                            start=(ot == 0), stop=(ot == N_IT - 1),
                        )
                m = kt - 2 * qb
                if m >= 0:  # one of the two diagonal tiles: additive mask
                    nc.vector.tensor_add(st[:], st[:], masksT[:, m, :])
                e = pwork.tile([P, P], BF16, tag="e")
                nc.scalar.activation(
                    e[:], st[:], mybir.ActivationFunctionType.Exp, scale=scale_eff
                )
                return e

            def emit_pv_phase(qb, kt, e):
                last = kt == 2 * qb + 1
                out2 = state[qb]["out2"]
                den = state[qb]["den"]
                for oc in range(D // KC):
                    nc.tensor.matmul(
                        out2[:, oc * KC:(oc + 1) * KC],
                        e[:], v_sb[:, kt, oc * KC:(oc + 1) * KC],
                        start=(kt == 0), stop=last,
                    )
                nc.tensor.matmul(den[:], e[:], ones_sb[:], start=(kt == 0), stop=last)
                if last:
                    linv = small.tile([P, 1], F32, tag="linv")
                    nc.vector.reciprocal(linv[:], den[:])
                    oh = small.tile([P, D], BF16, tag="oh")
                    # split the divide across ACT and DVE (GPSIMD cannot read
                    # PSUM) and store each half as soon as its engine drains it
                    nc.scalar.activation(
                        oh[:, 0:HD], out2[:, 0:HD],
                        mybir.ActivationFunctionType.Copy, scale=linv[:],
                    )
                    nc.sync.dma_start(
                        out[qb * P:(qb + 1) * P, 0:HD], oh[:, 0:HD]
                    )
                    nc.vector.tensor_scalar_mul(
                        oh[:, HD:D], out2[:, HD:D], linv[:]
                    )
                    nc.sync.dma_start(
                        out[qb * P:(qb + 1) * P, HD:D], oh[:, HD:D]
                    )
                    del state[qb]

            prev = None
            for qb, kt in tiles:
                e = emit_st_phase(qb, kt)
                if prev is not None:
                    emit_pv_phase(*prev)
                prev = (qb, kt, e)
            emit_pv_phase(*prev)

    return nc


_NC = {}


def _get_nc(fp8_proj: bool = FP8_PROJ):
    key = fp8_proj
    if key not in _NC:
        _NC[key] = build_nc(fp8_proj)
    return _NC[key]


def _prep_in_maps(inputs, Wq, Wk, Wv, fp8_proj: bool = FP8_PROJ):
    inputs = np.asarray(inputs, dtype=np.float32)
    Wq = np.asarray(Wq, dtype=np.float32)
    Wk = np.asarray(Wk, dtype=np.float32)
    Wv = np.asarray(Wv, dtype=np.float32)

    bf = ml_dtypes.bfloat16
    qk = ml_dtypes.float8_e4m3 if fp8_proj else bf
    ws = np.float32(W_SCALE) if fp8_proj else np.float32(1.0)
    wqT = np.ascontiguousarray(Wq.T * ws).astype(qk)
    wkT = np.ascontiguousarray(Wk.T * ws).astype(qk)
    wvT = np.ascontiguousarray(Wv.T).astype(bf)

    in_maps = []
    for c in range(NCORES):
        b, r = c // 2, c % 2
        xb = inputs[b]                                  # [T, D]
        xhTf = np.ascontiguousarray(xb.T)
        xhT = xhTf.astype(qk)
        xqT = np.ascontiguousarray(xb[r::2, :].T).astype(qk)  # [D, JQ]
        # additive causal mask for the two diagonal 128-key tiles of
        # each query block: S^T layout [key ks, query j], q_glob = 256*qb+2j+r,
        # key = 256*qb + 128*m + ks -> keep iff 128*m + ks <= 2j + r
        ks_idx = np.arange(P)[:, None]
        j_idx = np.arange(P)[None, :]
        masks = np.empty((2, P, P), dtype=np.float32)
        for m in range(2):
            keep = 128 * m + ks_idx <= 2 * j_idx + r
            masks[m] = np.where(keep, 0.0, MASK_NEG)
        im = {
            "xhT": xhT, "xqT": xqT,
            "wqT": wqT, "wkT": wkT, "wvT": wvT,
            "maskadd": masks,
        }
        if fp8_proj:
            im["xhTb"] = xhTf.astype(bf)
        in_maps.append(im)
    return in_maps


def _gather(res):
    result = np.empty((B, T, D), dtype=np.float32)
    for c in range(NCORES):
        b, r = c // 2, c % 2
        result[b, r::2, :] = res.results[c]["out"].astype(np.float32)
    return result


def kernel(inputs, Wq, Wk, Wv):
    in_maps = _prep_in_maps(inputs, Wq, Wk, Wv)
    nc = _get_nc()
    res = bass_utils.run_bass_kernel_spmd(nc, in_maps, core_ids=list(range(NCORES)))
    return _gather(res)


def run_traced(inputs, Wq, Wk, Wv):
    """Like kernel() but with NTFF tracing; returns BassKernelResults
    (exec_time_ns, trace path). For test.py only."""
    in_maps = _prep_in_maps(inputs, Wq, Wk, Wv)
    nc = _get_nc()
    res = bass_utils.run_bass_kernel_spmd(
        nc, in_maps, core_ids=list(range(NCORES)), trace=True
    )
    res.full_output = _gather(res)
    return res
